# revision 11
# baseline (speedup 1.0000x reference)
"""Multi-head attention (B=4, T=2048, D=1024, H=16) on 8 Trainium2 cores.

Sharding: batch (4-way) x head-half (2-way) -> 8 cores.
Core c handles batch b = c//2 and heads g*8..g*8+8 where g = c%2.

Per-core device program (all matmuls fp32r, N=512):
  Phase 1 (QKV): qkT[j,t] = Wqk^T-stationary GEMM over xT chunks (j on
    partitions, 8 j-tiles = [q 512 | k 512]); v[t,j] natural orientation,
    stored with a ones column appended per head ([t, h, 65]).
    Biases added via K=1 rank-1 matmuls into the psum accumulation.
  Phase 2 (attention, per head): scoresT[ki,qi] = kT^T q (K=64 contraction),
    DVE copy psum->sbuf, one wide exp (scale=1/8) per (h,ki-tile) on ACT,
    AV: oT[j,qi] accumulated over ki with lhsT=[v_h|ones] (M=65; row 64
    accumulates the softmax denominator). Normalize: reciprocal of row 64,
    DMA partition-broadcast, DVE multiply into ot[j,t] (f32r).
  Phase 3 (out-proj): out[t,c] = ot^T @ woT accumulated over 4 j-tiles.

Host: transposes/reshapes inputs per core, sums the two head-half partial
outputs per batch, adds out_b.
"""

import numpy as np
from contextlib import ExitStack

import concourse.bass as bass
import concourse.tile as tile
from concourse import bacc, mybir
from concourse.bass_utils import run_bass_kernel_spmd

B, T, D = 4, 2048, 1024
H, HD = 16, 64
P = 128
NC = 8
HPC = 8          # heads per core
JC = HPC * HD    # 512 head-dim columns per core
KT = D // P      # 8 contraction tiles for QKV
TT = T // P      # 16 t tiles
TCH = T // 512   # 4 t chunks of 512
F32 = mybir.dt.float32
F32R = mybir.dt.float32r

_cached = {}


def build_program():
    nc = bacc.Bacc("TRN2", target_bir_lowering=False, debug=False,
                   enable_asserts=True, num_devices=NC)

    xt_d = nc.dram_tensor("xt", [P, KT, T], F32R, kind="ExternalInput").ap()
    wqk_d = nc.dram_tensor("wqk", [P, KT, 2 * JC], F32R, kind="ExternalInput").ap()
    wv_d = nc.dram_tensor("wv", [P, KT, JC], F32R, kind="ExternalInput").ap()
    bqk_d = nc.dram_tensor("bqk", [1, 2 * JC], F32R, kind="ExternalInput").ap()
    bv_d = nc.dram_tensor("bv", [1, JC], F32R, kind="ExternalInput").ap()
    ones_d = nc.dram_tensor("ones", [P, 512], F32R, kind="ExternalInput").ap()
    wo_d = nc.dram_tensor("wo", [P, JC // P, D], F32R, kind="ExternalInput").ap()
    out_d = nc.dram_tensor("out", [T, D], F32, kind="ExternalOutput").ap()

    with tile.TileContext(nc) as tc:
        with ExitStack() as ctx:
            persist = ctx.enter_context(tc.tile_pool(name="persist", bufs=1))
            # persistent tiles
            qk_sb = persist.tile([P, 2 * JC // P, T], F32R, tag="qk")     # [128, 8, 2048]
            vaug = persist.tile([P, TT, HPC, HD + 1], F32R, tag="vaug")   # [128, 16, 8, 65]
            ones2d = persist.tile([P, 512], F32R, tag="ones2d")
            bqk_sb = persist.tile([1, 2 * JC], F32R, tag="bqk")
            bv_sb = persist.tile([1, JC], F32R, tag="bv")

            nc.sync.dma_start(ones2d[:], ones_d[:])
            ones_sb = ones2d[0:1, :]
            # ones column per head (vaug[..., 64]); v-GEMM copies fill [0:64]
            nc.sync.dma_start(
                vaug[:, :, :, HD:HD + 1],
                ones_d[:, 0:TT * HPC].rearrange(
                    "p (a b) -> p a b", a=TT)[:, :, :, None])
            nc.sync.dma_start(bqk_sb[:], bqk_d[:])
            nc.sync.dma_start(bv_sb[:], bv_d[:])

            # ---------------- Phase 1: QKV projections ----------------
            with ExitStack() as c1:
                with nc.named_scope("qkv"):
                    wpool = c1.enter_context(tc.tile_pool(name="wpool", bufs=1))
                    xpool = c1.enter_context(tc.tile_pool(name="xpool", bufs=2))
                    ps1 = c1.enter_context(
                        tc.tile_pool(name="ps1", bufs=2, space="PSUM"))
                    ps1v = c1.enter_context(
                        tc.tile_pool(name="ps1v", bufs=2, space="PSUM"))

                    wqk_sb = wpool.tile([P, KT, 2 * JC], F32R, tag="wqk")
                    wv_sb = wpool.tile([P, KT, JC], F32R, tag="wv")
                    nc.sync.dma_start(wqk_sb[:], wqk_d[:])
                    nc.sync.dma_start(wv_sb[:], wv_d[:])

                    for tci in range(TCH):
                        tsl = slice(tci * 512, (tci + 1) * 512)
                        xt = xpool.tile([P, KT, 512], F32R, tag="xt")
                        nc.sync.dma_start(xt[:], xt_d[:, :, tsl])

                        # qkT[j, t]: 8 j-tiles
                        for j in range(2 * JC // P):
                            ps = ps1.tile([P, 512], F32, tag="psq")
                            for k in range(KT):
                                nc.tensor.matmul(
                                    ps[:],
                                    wqk_sb[:, k, j * P:(j + 1) * P],
                                    xt[:, k, :],
                                    start=(k == 0), stop=False)
                            nc.tensor.matmul(
                                ps[:],
                                bqk_sb[0:1, j * P:(j + 1) * P],
                                ones_sb[0:1, :],
                                start=False, stop=True)
                            nc.vector.tensor_copy(qk_sb[:, j, tsl], ps[:])

                        # v[t, j] with per-head stride 65 in vaug
                        for tt in range(4):
                            tglob = tci * 4 + tt
                            ps = ps1v.tile([P, 512], F32, tag="psv")
                            for k in range(KT):
                                nc.tensor.matmul(
                                    ps[:],
                                    xt[:, k, tt * P:(tt + 1) * P],
                                    wv_sb[:, k, :],
                                    start=(k == 0), stop=False)
                            nc.tensor.matmul(
                                ps[:],
                                ones_sb[0:1, 0:P],
                                bv_sb[0:1, :],
                                start=False, stop=True)
                            nc.vector.tensor_copy(
                                vaug[:, tglob, :, 0:HD],
                                ps[:].rearrange("p (h d) -> p h d", h=HPC))

            # ot lives in phases 2+3 only (SBUF too small to coexist with
            # the phase-1 weight/x pools)
            otpool = ctx.enter_context(tc.tile_pool(name="otpool", bufs=1))
            ot_sb = otpool.tile([P, JC // P, T], F32R, tag="ot")          # [128, 4, 2048]

            # ---------------- Phase 2: attention per head ----------------
            with ExitStack() as c2:
                with nc.named_scope("attn"):
                    spool = c2.enter_context(tc.tile_pool(name="spool", bufs=2))
                    wtpool = c2.enter_context(tc.tile_pool(name="wtpool", bufs=3))
                    rcpool = c2.enter_context(tc.tile_pool(name="rcpool", bufs=2))
                    rbpool = c2.enter_context(tc.tile_pool(name="rbpool", bufs=3))
                    rdpool = c2.enter_context(
                        tc.tile_pool(name="rdpool", bufs=3, space="DRAM"))
                    pss = c2.enter_context(
                        tc.tile_pool(name="pss", bufs=2, space="PSUM"))
                    psav = c2.enter_context(
                        tc.tile_pool(name="psav", bufs=6, space="PSUM"))

                    for h in range(HPC):
                        pb = (h % 2) * 64          # partition base of this head
                        jt = h // 2                # j-tile (q); k is jt+4
                        qT = qk_sb[pb:pb + HD, jt, :]          # [64, 2048]
                        kTt = qk_sb[pb:pb + HD, jt + 4, :]     # [64, 2048]

                        av_tiles = [psav.tile([HD + 1, 512], F32, tag="av",
                                              name=f"av_{h}_{i}")
                                    for i in range(4)]
                        for k in range(TT):
                            s_sb = spool.tile([P, T], F32, tag="s")
                            for c4 in range(4):
                                ps = pss.tile([P, 512], F32, tag="ps_s")
                                nc.tensor.matmul(
                                    ps[:],
                                    kTt[:, k * P:(k + 1) * P],
                                    qT[:, c4 * 512:(c4 + 1) * 512],
                                    start=True, stop=True)
                                nc.vector.tensor_copy(
                                    s_sb[:, c4 * 512:(c4 + 1) * 512], ps[:])
                            wt = wtpool.tile([P, T], F32R, tag="wt")
                            nc.scalar.activation(
                                wt[:], s_sb[:],
                                mybir.ActivationFunctionType.Exp,
                                bias=0.0, scale=0.125)
                            for c4 in range(4):
                                nc.tensor.matmul(
                                    av_tiles[c4][:],
                                    vaug[:, k, h, :],
                                    wt[:, c4 * 512:(c4 + 1) * 512],
                                    start=(k == 0), stop=(k == TT - 1))

                        for c4 in range(4):
                            csl = slice(c4 * 512, (c4 + 1) * 512)
                            rc = rcpool.tile([P, 512], F32, tag="rc")
                            nc.vector.reciprocal(
                                rc[64:65, :], av_tiles[c4][HD:HD + 1, :])
                            rd = rdpool.tile([1, 512], F32, tag="rd")
                            nc.sync.dma_start(rd[:], rc[64:65, :])
                            rb = rbpool.tile([64, 512], F32, tag="rb")
                            nc.sync.dma_start(
                                rb[:], rd[:].to_broadcast((64, 512)))
                            nc.vector.tensor_mul(
                                ot_sb[pb:pb + 64, jt, csl],
                                av_tiles[c4][0:HD, :],
                                rb[:])

            # ---------------- Phase 3: out projection ----------------
            with ExitStack() as c3:
                with nc.named_scope("outproj"):
                    wopool = c3.enter_context(tc.tile_pool(name="wopool", bufs=1))
                    opool = c3.enter_context(tc.tile_pool(name="opool", bufs=3))
                    ps3 = c3.enter_context(
                        tc.tile_pool(name="ps3", bufs=4, space="PSUM"))

                    wo_sb = wopool.tile([P, JC // P, D], F32R, tag="wo")
                    nc.sync.dma_start(wo_sb[:], wo_d[:])

                    for tt in range(TT):
                        ost = opool.tile([P, D], F32, tag="ost")
                        for cc in range(2):
                            ps = ps3.tile([P, 512], F32, tag="pso")
                            for jt in range(JC // P):
                                nc.tensor.matmul(
                                    ps[:],
                                    ot_sb[:, jt, tt * P:(tt + 1) * P],
                                    wo_sb[:, jt, cc * 512:(cc + 1) * 512],
                                    start=(jt == 0), stop=(jt == JC // P - 1))
                            nc.vector.tensor_copy(
                                ost[:, cc * 512:(cc + 1) * 512], ps[:])
                        nc.sync.dma_start(out_d[tt * P:(tt + 1) * P, :], ost[:])

    nc.compile()
    return nc


def _prep_core_inputs(x, qkv_w, qkv_b, out_w, core):
    b, g = core // 2, core % 2
    jsl = slice(g * JC, (g + 1) * JC)

    xT = np.ascontiguousarray(x[b].T)                       # [1024, 2048]
    xt = np.ascontiguousarray(
        xT.reshape(KT, P, T).transpose(1, 0, 2))            # [128, 8, 2048]

    Wq = qkv_w[0 * D:1 * D][jsl]                            # [512, 1024]
    Wk = qkv_w[1 * D:2 * D][jsl]
    Wv = qkv_w[2 * D:3 * D][jsl]
    WqkT = np.concatenate([Wq, Wk], axis=0).T               # [1024, 1024]
    wqk = np.ascontiguousarray(
        WqkT.reshape(KT, P, 2 * JC).transpose(1, 0, 2))     # [128, 8, 1024]
    WvT = Wv.T                                              # [1024, 512]
    wv = np.ascontiguousarray(
        WvT.reshape(KT, P, JC).transpose(1, 0, 2))          # [128, 8, 512]

    bqk = np.concatenate(
        [qkv_b[0 * D:1 * D][jsl], qkv_b[1 * D:2 * D][jsl]])[None, :]
    bv = qkv_b[2 * D:3 * D][jsl][None, :]

    WoT = np.ascontiguousarray(out_w[:, jsl].T)             # [512, 1024]
    wo = np.ascontiguousarray(
        WoT.reshape(JC // P, P, D).transpose(1, 0, 2))      # [128, 4, 1024]

    return {
        "xt": xt.astype(np.float32),
        "wqk": wqk.astype(np.float32),
        "wv": wv.astype(np.float32),
        "bqk": bqk.astype(np.float32),
        "bv": bv.astype(np.float32),
        "wo": wo.astype(np.float32),
        "ones": np.ones((P, 512), dtype=np.float32),
    }


def run(x, qkv_w, qkv_b, out_w, out_b, trace=False, tmpdir=None):
    if "nc" not in _cached:
        _cached["nc"] = build_program()
    nc = _cached["nc"]
    in_maps = [_prep_core_inputs(x, qkv_w, qkv_b, out_w, c) for c in range(NC)]
    res = run_bass_kernel_spmd(nc, in_maps, core_ids=list(range(NC)),
                               trace=trace, tmpdir=tmpdir)
    parts = np.stack([res.results[c]["out"] for c in range(NC)])  # [8, T, D]
    out = parts.reshape(B, 2, T, D).sum(axis=1) + out_b[None, None, :]
    return out.astype(np.float32), res


def kernel(x, qkv_w, qkv_b, out_w, out_b):
    x = np.asarray(x, dtype=np.float32)
    qkv_w = np.asarray(qkv_w, dtype=np.float32)
    qkv_b = np.asarray(qkv_b, dtype=np.float32)
    out_w = np.asarray(out_w, dtype=np.float32)
    out_b = np.asarray(out_b, dtype=np.float32)
    out, _ = run(x, qkv_w, qkv_b, out_w, out_b, trace=False)
    return out


# revision 12
# speedup vs baseline: 1.1077x; 1.1077x over previous
"""Multi-head attention (B=4, T=2048, D=1024, H=16) on 8 Trainium2 cores.

Sharding: batch (4-way) x head-half (2-way) -> 8 cores.
Core c handles batch b = c//2 and heads g*8..g*8+8 where g = c%2.

Per-core device program (all matmuls fp32r, N=512):
  Phase 1 (QKV): qkT[j,t] = Wqk^T-stationary GEMM over xT chunks (j on
    partitions, 8 j-tiles = [q 512 | k 512]); v[t,j] natural orientation,
    stored with a ones column appended per head ([t, h, 65]).
    Biases added via K=1 rank-1 matmuls into the psum accumulation.
  Phase 2 (attention, per head): scoresT[ki,qi] = kT^T q (K=64 contraction),
    DVE copy psum->sbuf, one wide exp (scale=1/8) per (h,ki-tile) on ACT,
    AV: oT[j,qi] accumulated over ki with lhsT=[v_h|ones] (M=65; row 64
    accumulates the softmax denominator). Normalize: reciprocal of row 64,
    DMA partition-broadcast, DVE multiply into ot[j,t] (f32r).
  Phase 3 (out-proj): out[t,c] = ot^T @ woT accumulated over 4 j-tiles.

Host: transposes/reshapes inputs per core, sums the two head-half partial
outputs per batch, adds out_b.
"""

import numpy as np
from contextlib import ExitStack

import concourse.bass as bass
import concourse.tile as tile
from concourse import bacc, mybir
from concourse.bass_utils import run_bass_kernel_spmd

B, T, D = 4, 2048, 1024
H, HD = 16, 64
P = 128
NC = 8
HPC = 8          # heads per core
JC = HPC * HD    # 512 head-dim columns per core
KT = D // P      # 8 contraction tiles for QKV
TT = T // P      # 16 t tiles
TCH = T // 512   # 4 t chunks of 512
F32 = mybir.dt.float32
F32R = mybir.dt.float32r

_cached = {}


def build_program():
    nc = bacc.Bacc("TRN2", target_bir_lowering=False, debug=False,
                   enable_asserts=True, num_devices=NC)

    xt_d = nc.dram_tensor("xt", [P, KT, T], F32R, kind="ExternalInput").ap()
    wqk_d = nc.dram_tensor("wqk", [P, KT, 2 * JC], F32R, kind="ExternalInput").ap()
    wv_d = nc.dram_tensor("wv", [P, KT, JC], F32R, kind="ExternalInput").ap()
    bqk_d = nc.dram_tensor("bqk", [1, 2 * JC], F32R, kind="ExternalInput").ap()
    bv_d = nc.dram_tensor("bv", [1, JC], F32R, kind="ExternalInput").ap()
    ones_d = nc.dram_tensor("ones", [P, 512], F32R, kind="ExternalInput").ap()
    wo_d = nc.dram_tensor("wo", [P, JC // P, D], F32R, kind="ExternalInput").ap()
    out_d = nc.dram_tensor("out", [T, D], F32, kind="ExternalOutput").ap()

    with tile.TileContext(nc) as tc:
        with ExitStack() as ctx:
            persist = ctx.enter_context(tc.tile_pool(name="persist", bufs=1))
            # persistent tiles
            qk_sb = persist.tile([P, 2 * JC // P, T], F32R, tag="qk")     # [128, 8, 2048]
            vaug = persist.tile([P, TT, HPC, HD + 1], F32R, tag="vaug")   # [128, 16, 8, 65]
            ones2d = persist.tile([P, 512], F32R, tag="ones2d")
            bqk_sb = persist.tile([1, 2 * JC], F32R, tag="bqk")
            bv_sb = persist.tile([1, JC], F32R, tag="bv")

            nc.sync.dma_start(ones2d[:], ones_d[:])
            ones_sb = ones2d[0:1, :]
            # ones column per head (vaug[..., 64]); v-GEMM copies fill [0:64]
            nc.sync.dma_start(
                vaug[:, :, :, HD:HD + 1],
                ones_d[:, 0:TT * HPC].rearrange(
                    "p (a b) -> p a b", a=TT)[:, :, :, None])
            nc.sync.dma_start(bqk_sb[:], bqk_d[:])
            nc.sync.dma_start(bv_sb[:], bv_d[:])

            # ---------------- Phase 1: QKV projections ----------------
            with ExitStack() as c1:
                with nc.named_scope("qkv"):
                    wpool = c1.enter_context(tc.tile_pool(name="wpool", bufs=1))
                    xpool = c1.enter_context(tc.tile_pool(name="xpool", bufs=2))
                    ps1 = c1.enter_context(
                        tc.tile_pool(name="ps1", bufs=2, space="PSUM"))
                    ps1v = c1.enter_context(
                        tc.tile_pool(name="ps1v", bufs=2, space="PSUM"))

                    wqk_sb = wpool.tile([P, KT, 2 * JC], F32R, tag="wqk")
                    wv_sb = wpool.tile([P, KT, JC], F32R, tag="wv")
                    nc.sync.dma_start(wqk_sb[:], wqk_d[:])
                    nc.sync.dma_start(wv_sb[:], wv_d[:])

                    for tci in range(TCH):
                        tsl = slice(tci * 512, (tci + 1) * 512)
                        xt = xpool.tile([P, KT, 512], F32R, tag="xt")
                        nc.sync.dma_start(xt[:], xt_d[:, :, tsl])

                        # qkT[j, t]: 8 j-tiles
                        for j in range(2 * JC // P):
                            ps = ps1.tile([P, 512], F32, tag="psq")
                            for k in range(KT):
                                nc.tensor.matmul(
                                    ps[:],
                                    wqk_sb[:, k, j * P:(j + 1) * P],
                                    xt[:, k, :],
                                    start=(k == 0), stop=False)
                            nc.tensor.matmul(
                                ps[:],
                                bqk_sb[0:1, j * P:(j + 1) * P],
                                ones_sb[0:1, :],
                                start=False, stop=True)
                            nc.vector.tensor_copy(qk_sb[:, j, tsl], ps[:])

                        # v[t, j] with per-head stride 65 in vaug
                        for tt in range(4):
                            tglob = tci * 4 + tt
                            ps = ps1v.tile([P, 512], F32, tag="psv")
                            for k in range(KT):
                                nc.tensor.matmul(
                                    ps[:],
                                    xt[:, k, tt * P:(tt + 1) * P],
                                    wv_sb[:, k, :],
                                    start=(k == 0), stop=False)
                            nc.tensor.matmul(
                                ps[:],
                                ones_sb[0:1, 0:P],
                                bv_sb[0:1, :],
                                start=False, stop=True)
                            nc.vector.tensor_copy(
                                vaug[:, tglob, :, 0:HD],
                                ps[:].rearrange("p (h d) -> p h d", h=HPC))

            # ot lives in phases 2+3 only (SBUF too small to coexist with
            # the phase-1 weight/x pools)
            otpool = ctx.enter_context(tc.tile_pool(name="otpool", bufs=1))
            ot_sb = otpool.tile([P, JC // P, T], F32R, tag="ot")          # [128, 4, 2048]

            # ---------------- Phase 2: attention per head ----------------
            with ExitStack() as c2:
                with nc.named_scope("attn"):
                    wtpool = c2.enter_context(tc.tile_pool(name="wtpool", bufs=3))
                    rcpool = c2.enter_context(tc.tile_pool(name="rcpool", bufs=4))
                    rbpool = c2.enter_context(tc.tile_pool(name="rbpool", bufs=3))
                    rdpool = c2.enter_context(
                        tc.tile_pool(name="rdpool", bufs=3, space="DRAM"))
                    pss = c2.enter_context(
                        tc.tile_pool(name="pss", bufs=2, space="PSUM"))
                    psav = c2.enter_context(
                        tc.tile_pool(name="psav", bufs=4, space="PSUM"))

                    EXP = mybir.ActivationFunctionType.Exp
                    LN = mybir.ActivationFunctionType.Ln
                    for h in range(HPC):
                        pb = (h % 2) * 64          # partition base of this head
                        jt = h // 2                # j-tile (q); k is jt+4
                        qT = qk_sb[pb:pb + HD, jt, :]          # [64, 2048]
                        kTt = qk_sb[pb:pb + HD, jt + 4, :]     # [64, 2048]

                        av_tiles = [psav.tile([HD + 1, 512], F32, tag="av",
                                              name=f"av_{h}_{i}")
                                    for i in range(4)]
                        for k in range(TT):
                            wt = wtpool.tile([P, T], F32R, tag="wt")
                            # scores -> psum (2 chunks of 1024), exp straight
                            # from psum into sbuf (keeps DVE out of the chain)
                            for c2_ in range(2):
                                ps = pss.tile([P, 2, 512], F32, tag="ps_s")
                                for cc in range(2):
                                    c4 = c2_ * 2 + cc
                                    nc.tensor.matmul(
                                        ps[:, cc, :],
                                        kTt[:, k * P:(k + 1) * P],
                                        qT[:, c4 * 512:(c4 + 1) * 512],
                                        start=True, stop=True)
                                nc.scalar.activation(
                                    wt[:, c2_ * 1024:(c2_ + 1) * 1024],
                                    ps[:].rearrange("p a b -> p (a b)"),
                                    EXP, bias=0.0, scale=0.125)
                            for c4 in range(4):
                                nc.tensor.matmul(
                                    av_tiles[c4][:],
                                    vaug[:, k, h, :],
                                    wt[:, c4 * 512:(c4 + 1) * 512],
                                    start=(k == 0), stop=(k == TT - 1))

                        for c4 in range(4):
                            csl = slice(c4 * 512, (c4 + 1) * 512)
                            # 1/s = exp(-ln(s)) on ACT (DVE reciprocal on a
                            # single partition costs 3.3us)
                            rc = rcpool.tile([P, 512], F32, tag="rc")
                            nc.scalar.activation(
                                rc[64:65, :], av_tiles[c4][HD:HD + 1, :],
                                LN, bias=0.0, scale=1.0)
                            rc2 = rcpool.tile([P, 512], F32, tag="rc2")
                            nc.scalar.activation(
                                rc2[64:65, :], rc[64:65, :],
                                EXP, bias=0.0, scale=-1.0)
                            rd = rdpool.tile([1, 512], F32, tag="rd")
                            nc.sync.dma_start(rd[:], rc2[64:65, :])
                            rb = rbpool.tile([64, 512], F32, tag="rb")
                            nc.sync.dma_start(
                                rb[:], rd[:].to_broadcast((64, 512)))
                            nc.vector.tensor_mul(
                                ot_sb[pb:pb + 64, jt, csl],
                                av_tiles[c4][0:HD, :],
                                rb[:])

            # ---------------- Phase 3: out projection ----------------
            with ExitStack() as c3:
                with nc.named_scope("outproj"):
                    wopool = c3.enter_context(tc.tile_pool(name="wopool", bufs=1))
                    opool = c3.enter_context(tc.tile_pool(name="opool", bufs=3))
                    ps3 = c3.enter_context(
                        tc.tile_pool(name="ps3", bufs=4, space="PSUM"))

                    wo_sb = wopool.tile([P, JC // P, D], F32R, tag="wo")
                    nc.sync.dma_start(wo_sb[:], wo_d[:])

                    for tt in range(TT):
                        ost = opool.tile([P, D], F32, tag="ost")
                        for cc in range(2):
                            ps = ps3.tile([P, 512], F32, tag="pso")
                            for jt in range(JC // P):
                                nc.tensor.matmul(
                                    ps[:],
                                    ot_sb[:, jt, tt * P:(tt + 1) * P],
                                    wo_sb[:, jt, cc * 512:(cc + 1) * 512],
                                    start=(jt == 0), stop=(jt == JC // P - 1))
                            nc.vector.tensor_copy(
                                ost[:, cc * 512:(cc + 1) * 512], ps[:])
                        nc.sync.dma_start(out_d[tt * P:(tt + 1) * P, :], ost[:])

    nc.compile()
    return nc


def _prep_core_inputs(x, qkv_w, qkv_b, out_w, core):
    b, g = core // 2, core % 2
    jsl = slice(g * JC, (g + 1) * JC)

    xT = np.ascontiguousarray(x[b].T)                       # [1024, 2048]
    xt = np.ascontiguousarray(
        xT.reshape(KT, P, T).transpose(1, 0, 2))            # [128, 8, 2048]

    Wq = qkv_w[0 * D:1 * D][jsl]                            # [512, 1024]
    Wk = qkv_w[1 * D:2 * D][jsl]
    Wv = qkv_w[2 * D:3 * D][jsl]
    WqkT = np.concatenate([Wq, Wk], axis=0).T               # [1024, 1024]
    wqk = np.ascontiguousarray(
        WqkT.reshape(KT, P, 2 * JC).transpose(1, 0, 2))     # [128, 8, 1024]
    WvT = Wv.T                                              # [1024, 512]
    wv = np.ascontiguousarray(
        WvT.reshape(KT, P, JC).transpose(1, 0, 2))          # [128, 8, 512]

    bqk = np.concatenate(
        [qkv_b[0 * D:1 * D][jsl], qkv_b[1 * D:2 * D][jsl]])[None, :]
    bv = qkv_b[2 * D:3 * D][jsl][None, :]

    WoT = np.ascontiguousarray(out_w[:, jsl].T)             # [512, 1024]
    wo = np.ascontiguousarray(
        WoT.reshape(JC // P, P, D).transpose(1, 0, 2))      # [128, 4, 1024]

    return {
        "xt": xt.astype(np.float32),
        "wqk": wqk.astype(np.float32),
        "wv": wv.astype(np.float32),
        "bqk": bqk.astype(np.float32),
        "bv": bv.astype(np.float32),
        "wo": wo.astype(np.float32),
        "ones": np.ones((P, 512), dtype=np.float32),
    }


def run(x, qkv_w, qkv_b, out_w, out_b, trace=False, tmpdir=None):
    if "nc" not in _cached:
        _cached["nc"] = build_program()
    nc = _cached["nc"]
    in_maps = [_prep_core_inputs(x, qkv_w, qkv_b, out_w, c) for c in range(NC)]
    res = run_bass_kernel_spmd(nc, in_maps, core_ids=list(range(NC)),
                               trace=trace, tmpdir=tmpdir)
    parts = np.stack([res.results[c]["out"] for c in range(NC)])  # [8, T, D]
    out = parts.reshape(B, 2, T, D).sum(axis=1) + out_b[None, None, :]
    return out.astype(np.float32), res


def kernel(x, qkv_w, qkv_b, out_w, out_b):
    x = np.asarray(x, dtype=np.float32)
    qkv_w = np.asarray(qkv_w, dtype=np.float32)
    qkv_b = np.asarray(qkv_b, dtype=np.float32)
    out_w = np.asarray(out_w, dtype=np.float32)
    out_b = np.asarray(out_b, dtype=np.float32)
    out, _ = run(x, qkv_w, qkv_b, out_w, out_b, trace=False)
    return out


# revision 19
# speedup vs baseline: 1.1913x; 1.0755x over previous
"""Multi-head attention (B=4, T=2048, D=1024, H=16) on 8 Trainium2 cores.

Sharding: batch (4-way) x head-half (2-way) -> 8 cores.
Core c handles batch b = c//2 and heads g*8..g*8+8 where g = c%2.

Per-core device program (all matmuls fp32r, N=512):
  Phase 1 (QKV): qkT[j,t] = Wqk^T-stationary GEMM over xT chunks (j on
    partitions, 8 j-tiles = [q 512 | k 512]); v[t,j] natural orientation,
    stored with a ones column appended per head ([t, h, 65]).
    Biases added via K=1 rank-1 matmuls into the psum accumulation.
  Phase 2 (attention, per head): scoresT[ki,qi] = kT^T q (K=64 contraction),
    DVE copy psum->sbuf, one wide exp (scale=1/8) per (h,ki-tile) on ACT,
    AV: oT[j,qi] accumulated over ki with lhsT=[v_h|ones] (M=65; row 64
    accumulates the softmax denominator). Normalize: reciprocal of row 64,
    DMA partition-broadcast, DVE multiply into ot[j,t] (f32r).
  Phase 3 (out-proj): out[t,c] = ot^T @ woT accumulated over 4 j-tiles.

Host: transposes/reshapes inputs per core, sums the two head-half partial
outputs per batch, adds out_b.
"""

import numpy as np
from contextlib import ExitStack

import concourse.bass as bass
import concourse.tile as tile
from concourse import bacc, mybir
from concourse.bass_utils import run_bass_kernel_spmd

B, T, D = 4, 2048, 1024
H, HD = 16, 64
P = 128
NC = 8
HPC = 8          # heads per core
JC = HPC * HD    # 512 head-dim columns per core
KT = D // P      # 8 contraction tiles for QKV
TT = T // P      # 16 t tiles
TCH = T // 512   # 4 t chunks of 512
F32 = mybir.dt.float32
F32R = mybir.dt.float32r

_cached = {}


def build_program():
    nc = bacc.Bacc("TRN2", target_bir_lowering=False, debug=False,
                   enable_asserts=True, num_devices=NC)

    xt_d = nc.dram_tensor("xt", [P, KT, T], F32R, kind="ExternalInput").ap()
    wqk_d = nc.dram_tensor("wqk", [P, KT, 2 * JC], F32R, kind="ExternalInput").ap()
    wv_d = nc.dram_tensor("wv", [P, KT, JC], F32R, kind="ExternalInput").ap()
    bqk_d = nc.dram_tensor("bqk", [1, 2 * JC], F32R, kind="ExternalInput").ap()
    bv_d = nc.dram_tensor("bv", [1, JC], F32R, kind="ExternalInput").ap()
    ones_d = nc.dram_tensor("ones", [P, 512], F32R, kind="ExternalInput").ap()
    wo_d = nc.dram_tensor("wo", [P, JC // P, D], F32R, kind="ExternalInput").ap()
    out_d = nc.dram_tensor("out", [T, D], F32, kind="ExternalOutput").ap()

    with tile.TileContext(nc) as tc:
        with ExitStack() as ctx:
            persist = ctx.enter_context(tc.tile_pool(name="persist", bufs=1))
            # persistent tiles
            qk_sb = persist.tile([P, 2 * JC // P, T], F32R, tag="qk")     # [128, 8, 2048]
            vaug = persist.tile([P, TT, HPC, HD + 1], F32R, tag="vaug")   # [128, 16, 8, 65]
            ones2d = persist.tile([P, 512], F32R, tag="ones2d")
            bqk_sb = persist.tile([1, 2 * JC], F32R, tag="bqk")
            bv_sb = persist.tile([1, JC], F32R, tag="bv")

            nc.sync.dma_start(ones2d[:], ones_d[:])
            ones_sb = ones2d[0:1, :]
            # ones column per head (vaug[..., 64]); v-GEMM copies fill [0:64]
            nc.sync.dma_start(
                vaug[:, :, :, HD:HD + 1],
                ones_d[:, 0:TT * HPC].rearrange(
                    "p (a b) -> p a b", a=TT)[:, :, :, None])
            nc.sync.dma_start(bqk_sb[:], bqk_d[:])
            nc.sync.dma_start(bv_sb[:], bv_d[:])

            # ---------------- Phase 1: QKV projections ----------------
            with ExitStack() as c1:
                with nc.named_scope("qkv"):
                    wpool = c1.enter_context(tc.tile_pool(name="wpool", bufs=1))
                    xpool = c1.enter_context(tc.tile_pool(name="xpool", bufs=2))
                    ps1 = c1.enter_context(
                        tc.tile_pool(name="ps1", bufs=2, space="PSUM"))
                    ps1v = c1.enter_context(
                        tc.tile_pool(name="ps1v", bufs=2, space="PSUM"))

                    wqk_sb = wpool.tile([P, KT, 2 * JC], F32R, tag="wqk")
                    wv_sb = wpool.tile([P, KT, JC], F32R, tag="wv")
                    nc.sync.dma_start(wqk_sb[:], wqk_d[:])
                    nc.sync.dma_start(wv_sb[:], wv_d[:])

                    for tci in range(TCH):
                        tsl = slice(tci * 512, (tci + 1) * 512)
                        xt = xpool.tile([P, KT, 512], F32R, tag="xt")
                        nc.sync.dma_start(xt[:], xt_d[:, :, tsl])

                        # qkT[j, t]: 8 j-tiles
                        for j in range(2 * JC // P):
                            ps = ps1.tile([P, 512], F32, tag="psq")
                            for k in range(KT):
                                nc.tensor.matmul(
                                    ps[:],
                                    wqk_sb[:, k, j * P:(j + 1) * P],
                                    xt[:, k, :],
                                    start=(k == 0), stop=False)
                            nc.tensor.matmul(
                                ps[:],
                                bqk_sb[0:1, j * P:(j + 1) * P],
                                ones_sb[0:1, :],
                                start=False, stop=True)
                            nc.vector.tensor_copy(qk_sb[:, j, tsl], ps[:])

                        # v[t, j] with per-head stride 65 in vaug
                        for tt in range(4):
                            tglob = tci * 4 + tt
                            ps = ps1v.tile([P, 512], F32, tag="psv")
                            for k in range(KT):
                                nc.tensor.matmul(
                                    ps[:],
                                    xt[:, k, tt * P:(tt + 1) * P],
                                    wv_sb[:, k, :],
                                    start=(k == 0), stop=False)
                            nc.tensor.matmul(
                                ps[:],
                                ones_sb[0:1, 0:P],
                                bv_sb[0:1, :],
                                start=False, stop=True)
                            nc.vector.tensor_copy(
                                vaug[:, tglob, :, 0:HD],
                                ps[:].rearrange("p (h d) -> p h d", h=HPC))

            # ot lives in phases 2+3 only (SBUF too small to coexist with
            # the phase-1 weight/x pools)
            otpool = ctx.enter_context(tc.tile_pool(name="otpool", bufs=1))
            ot_sb = otpool.tile([P, JC // P, T], F32R, tag="ot")          # [128, 4, 2048]

            # ---------------- Phase 2: attention per head ----------------
            with ExitStack() as c2:
                with nc.named_scope("attn"):
                    wtpool = c2.enter_context(tc.tile_pool(name="wtpool", bufs=3))
                    nrmpool = c2.enter_context(tc.tile_pool(name="nrmpool", bufs=2))
                    rbpool = c2.enter_context(tc.tile_pool(name="rbpool", bufs=1))
                    rdpool = c2.enter_context(
                        tc.tile_pool(name="rdpool", bufs=2, space="DRAM"))
                    pss = c2.enter_context(
                        tc.tile_pool(name="pss", bufs=1, space="PSUM"))
                    psav = c2.enter_context(
                        tc.tile_pool(name="psav", bufs=4, space="PSUM"))

                    EXP = mybir.ActivationFunctionType.Exp

                    def do_scores(h, k, av_tiles):
                        pb = (h % 2) * 64
                        jt = h // 2
                        qT = qk_sb[pb:pb + HD, jt, :]
                        kTt = qk_sb[pb:pb + HD, jt + 4, :]
                        wt = wtpool.tile([P, T], F32R, tag="wt",
                                         name=f"wt_{h}_{k}")
                        ps = pss.tile([P, 4, 512], F32, tag="ps_s",
                                      name=f"ps_s_{h}_{k}")
                        for c4 in range(4):
                            nc.tensor.matmul(
                                ps[:, c4, :],
                                kTt[:, k * P:(k + 1) * P],
                                qT[:, c4 * 512:(c4 + 1) * 512],
                                start=True, stop=True)
                        # one maximally wide exp straight from psum
                        nc.scalar.activation(
                            wt[:], ps[:].rearrange("p a b -> p (a b)"),
                            EXP, bias=0.0, scale=0.125)
                        return wt

                    def do_av(h, k, wt, av_tiles):
                        for c4 in range(4):
                            nc.tensor.matmul(
                                av_tiles[c4][:],
                                vaug[:, k, h, :],
                                wt[:, c4 * 512:(c4 + 1) * 512],
                                start=(k == 0), stop=(k == TT - 1))

                    def finish_head(h, av_tiles):
                        pb = (h % 2) * 64
                        jt = h // 2
                        # free psum fast: copy o rows (unnormalized) + sums row
                        # (sums collect at partition 64, aligned with psum)
                        sums = nrmpool.tile([P, T], F32, tag="sums",
                                            name=f"sums_{h}")
                        for c4 in range(4):
                            csl = slice(c4 * 512, (c4 + 1) * 512)
                            nc.vector.tensor_copy(
                                ot_sb[pb:pb + 64, jt, csl],
                                av_tiles[c4][0:HD, :])
                            nc.vector.tensor_copy(
                                sums[64:65, csl],
                                av_tiles[c4][HD:HD + 1, :])
                        nc.vector.reciprocal(sums[64:65, :], sums[64:65, :])
                        rd = rdpool.tile([1, T], F32, tag="rd",
                                         name=f"rd_{h}")
                        nc.sync.dma_start(rd[:], sums[64:65, :])
                        rb = rbpool.tile([P, T], F32, tag="rb",
                                         name=f"rb_{h}")
                        nc.sync.dma_start(rb[pb:pb + 64, :],
                                          rd[:].to_broadcast((64, T)))
                        nc.vector.tensor_mul(
                            ot_sb[pb:pb + 64, jt, :],
                            ot_sb[pb:pb + 64, jt, :],
                            rb[pb:pb + 64, :])

                    # software pipeline: AV(k-1) is emitted after scores(k) so
                    # the PE always has independent matmuls between exp waits
                    prev = None
                    for h in range(HPC):
                        av_tiles = [psav.tile([HD + 1, 512], F32, tag="av",
                                              name=f"av_{h}_{i}")
                                    for i in range(4)]
                        for k in range(TT):
                            wt = do_scores(h, k, av_tiles)
                            if prev is not None:
                                ph, pk, pwt, pav = prev
                                do_av(ph, pk, pwt, pav)
                                if pk == TT - 1:
                                    finish_head(ph, pav)
                            prev = (h, k, wt, av_tiles)
                    ph, pk, pwt, pav = prev
                    do_av(ph, pk, pwt, pav)
                    finish_head(ph, pav)

            # ---------------- Phase 3: out projection ----------------
            with ExitStack() as c3:
                with nc.named_scope("outproj"):
                    wopool = c3.enter_context(tc.tile_pool(name="wopool", bufs=1))
                    opool = c3.enter_context(tc.tile_pool(name="opool", bufs=3))
                    ps3 = c3.enter_context(
                        tc.tile_pool(name="ps3", bufs=4, space="PSUM"))

                    wo_sb = wopool.tile([P, JC // P, D], F32R, tag="wo")
                    nc.sync.dma_start(wo_sb[:], wo_d[:])

                    for tt in range(TT):
                        ost = opool.tile([P, D], F32, tag="ost")
                        for cc in range(2):
                            ps = ps3.tile([P, 512], F32, tag="pso")
                            for jt in range(JC // P):
                                nc.tensor.matmul(
                                    ps[:],
                                    ot_sb[:, jt, tt * P:(tt + 1) * P],
                                    wo_sb[:, jt, cc * 512:(cc + 1) * 512],
                                    start=(jt == 0), stop=(jt == JC // P - 1))
                            nc.vector.tensor_copy(
                                ost[:, cc * 512:(cc + 1) * 512], ps[:])
                        nc.sync.dma_start(out_d[tt * P:(tt + 1) * P, :], ost[:])

    nc.compile()
    return nc


def _prep_core_inputs(x, qkv_w, qkv_b, out_w, core):
    b, g = core // 2, core % 2
    jsl = slice(g * JC, (g + 1) * JC)

    xT = np.ascontiguousarray(x[b].T)                       # [1024, 2048]
    xt = np.ascontiguousarray(
        xT.reshape(KT, P, T).transpose(1, 0, 2))            # [128, 8, 2048]

    Wq = qkv_w[0 * D:1 * D][jsl]                            # [512, 1024]
    Wk = qkv_w[1 * D:2 * D][jsl]
    Wv = qkv_w[2 * D:3 * D][jsl]
    WqkT = np.concatenate([Wq, Wk], axis=0).T               # [1024, 1024]
    wqk = np.ascontiguousarray(
        WqkT.reshape(KT, P, 2 * JC).transpose(1, 0, 2))     # [128, 8, 1024]
    WvT = Wv.T                                              # [1024, 512]
    wv = np.ascontiguousarray(
        WvT.reshape(KT, P, JC).transpose(1, 0, 2))          # [128, 8, 512]

    bqk = np.concatenate(
        [qkv_b[0 * D:1 * D][jsl], qkv_b[1 * D:2 * D][jsl]])[None, :]
    bv = qkv_b[2 * D:3 * D][jsl][None, :]

    WoT = np.ascontiguousarray(out_w[:, jsl].T)             # [512, 1024]
    wo = np.ascontiguousarray(
        WoT.reshape(JC // P, P, D).transpose(1, 0, 2))      # [128, 4, 1024]

    return {
        "xt": xt.astype(np.float32),
        "wqk": wqk.astype(np.float32),
        "wv": wv.astype(np.float32),
        "bqk": bqk.astype(np.float32),
        "bv": bv.astype(np.float32),
        "wo": wo.astype(np.float32),
        "ones": np.ones((P, 512), dtype=np.float32),
    }


def run(x, qkv_w, qkv_b, out_w, out_b, trace=False, tmpdir=None):
    if "nc" not in _cached:
        _cached["nc"] = build_program()
    nc = _cached["nc"]
    in_maps = [_prep_core_inputs(x, qkv_w, qkv_b, out_w, c) for c in range(NC)]
    res = run_bass_kernel_spmd(nc, in_maps, core_ids=list(range(NC)),
                               trace=trace, tmpdir=tmpdir)
    parts = np.stack([res.results[c]["out"] for c in range(NC)])  # [8, T, D]
    out = parts.reshape(B, 2, T, D).sum(axis=1) + out_b[None, None, :]
    return out.astype(np.float32), res


def kernel(x, qkv_w, qkv_b, out_w, out_b):
    x = np.asarray(x, dtype=np.float32)
    qkv_w = np.asarray(qkv_w, dtype=np.float32)
    qkv_b = np.asarray(qkv_b, dtype=np.float32)
    out_w = np.asarray(out_w, dtype=np.float32)
    out_b = np.asarray(out_b, dtype=np.float32)
    out, _ = run(x, qkv_w, qkv_b, out_w, out_b, trace=False)
    return out


# revision 27
# speedup vs baseline: 1.4056x; 1.1798x over previous
"""Multi-head attention (B=4, T=2048, D=1024, H=16) on 8 Trainium2 cores.

Sharding: batch (4-way) x head-half (2-way) -> 8 cores.
Core c handles batch b = c//2 and heads g*8..g*8+8 where g = c%2.

Per-core device program (all matmuls fp32r, N=512):
  Phase 1 (QKV): qkT[j,t] = Wqk^T-stationary GEMM over xT chunks (j on
    partitions, 8 j-tiles = [q 512 | k 512]); v[t,j] natural orientation,
    stored with a ones column appended per head ([t, h, 65]).
    Biases added via K=1 rank-1 matmuls into the psum accumulation.
  Phase 2 (attention, per head): scoresT[ki,qi] = kT^T q (K=64 contraction),
    DVE copy psum->sbuf, one wide exp (scale=1/8) per (h,ki-tile) on ACT,
    AV: oT[j,qi] accumulated over ki with lhsT=[v_h|ones] (M=65; row 64
    accumulates the softmax denominator). Normalize: reciprocal of row 64,
    DMA partition-broadcast, DVE multiply into ot[j,t] (f32r).
  Phase 3 (out-proj): out[t,c] = ot^T @ woT accumulated over 4 j-tiles.

Host: transposes/reshapes inputs per core, sums the two head-half partial
outputs per batch, adds out_b.
"""

import numpy as np
from contextlib import ExitStack

import concourse.bass as bass
import concourse.tile as tile
from concourse import bacc, mybir
from concourse.bass_utils import run_bass_kernel_spmd

B, T, D = 4, 2048, 1024
H, HD = 16, 64
P = 128
NC = 8
HPC = 8          # heads per core
JC = HPC * HD    # 512 head-dim columns per core
KT = D // P      # 8 contraction tiles for QKV
TT = T // P      # 16 t tiles
TCH = T // 512   # 4 t chunks of 512
F32 = mybir.dt.float32
F32R = mybir.dt.float32r

_cached = {}


def build_program():
    nc = bacc.Bacc("TRN2", target_bir_lowering=False, debug=False,
                   enable_asserts=True, num_devices=NC)

    xt_d = nc.dram_tensor("xt", [P, KT, T], F32R, kind="ExternalInput").ap()
    wqk_d = nc.dram_tensor("wqk", [P, KT, 2 * JC], F32R, kind="ExternalInput").ap()
    wv_d = nc.dram_tensor("wv", [P, KT, JC], F32R, kind="ExternalInput").ap()
    bqk_d = nc.dram_tensor("bqk", [1, 2 * JC], F32R, kind="ExternalInput").ap()
    bv_d = nc.dram_tensor("bv", [1, JC], F32R, kind="ExternalInput").ap()
    ones_d = nc.dram_tensor("ones", [P, 512], F32R, kind="ExternalInput").ap()
    zeros_d = nc.dram_tensor("zeros", [P, T], F32R, kind="ExternalInput").ap()
    wo_d = nc.dram_tensor("wo", [P, JC // P, D], F32R, kind="ExternalInput").ap()
    out_d = nc.dram_tensor("out", [T, D], F32, kind="ExternalOutput").ap()

    with tile.TileContext(nc) as tc:
        with ExitStack() as ctx:
            persist = ctx.enter_context(tc.tile_pool(name="persist", bufs=1))
            # persistent tiles
            qk_sb = persist.tile([P, 2 * JC // P, T], F32R, tag="qk")     # [128, 8, 2048]
            # per t-tile: 8 heads x [v(64) | ones(1)] + 64 pad columns so the
            # AV stationary operand can always be sliced 128 wide (M=128 keeps
            # the PE array activity monitor from downclocking)
            VW = HPC * (HD + 1)
            vaug_f = persist.tile([P, TT, VW + HD], F32R, tag="vaug")
            ones2d = persist.tile([P, 512], F32R, tag="ones2d")
            bqk_sb = persist.tile([1, 2 * JC], F32R, tag="bqk")
            bv_sb = persist.tile([1, JC], F32R, tag="bv")

            nc.sync.dma_start(ones2d[:], ones_d[:])
            ones_sb = ones2d[0:1, :]
            vaug = vaug_f[:, :, 0:VW].rearrange(
                "p t (h e) -> p t h e", h=HPC)          # [128, 16, 8, 65]
            # ones column per head (vaug[..., 64]); v-GEMM copies fill [0:64]
            for tt in range(TT):
                nc.sync.dma_start(
                    vaug[:, tt, :, HD:HD + 1],
                    ones_d[:, 0:HPC, None])
            # fill the pad region with ones too (never read as real data, but
            # must be finite for the padded AV matmuls)
            nc.sync.dma_start(
                vaug_f[:, :, VW:VW + HD],
                ones_d[:, None, 0:HD].to_broadcast((P, TT, HD)))
            nc.sync.dma_start(bqk_sb[:], bqk_d[:])
            nc.sync.dma_start(bv_sb[:], bv_d[:])

            # ---------------- Phase 1: QKV projections ----------------
            with ExitStack() as c1:
                with nc.named_scope("qkv"):
                    wpool = c1.enter_context(tc.tile_pool(name="wpool", bufs=1))
                    xpool = c1.enter_context(tc.tile_pool(name="xpool", bufs=2))
                    ps1 = c1.enter_context(
                        tc.tile_pool(name="ps1", bufs=2, space="PSUM"))
                    ps1v = c1.enter_context(
                        tc.tile_pool(name="ps1v", bufs=2, space="PSUM"))

                    wqk_sb = wpool.tile([P, KT, 2 * JC], F32R, tag="wqk")
                    wv_sb = wpool.tile([P, KT, JC], F32R, tag="wv")
                    for k in range(KT):
                        nc.sync.dma_start(wqk_sb[:, k, :], wqk_d[:, k, :])
                        nc.sync.dma_start(wv_sb[:, k, :], wv_d[:, k, :])

                    for tci in range(TCH):
                        tsl = slice(tci * 512, (tci + 1) * 512)
                        xt = xpool.tile([P, KT, 512], F32R, tag="xt")
                        nc.sync.dma_start(xt[:], xt_d[:, :, tsl])

                        # qkT[j, t]: 8 j-tiles
                        for j in range(2 * JC // P):
                            ps = ps1.tile([P, 512], F32, tag="psq")
                            for k in range(KT):
                                nc.tensor.matmul(
                                    ps[:],
                                    wqk_sb[:, k, j * P:(j + 1) * P],
                                    xt[:, k, :],
                                    start=(k == 0), stop=False)
                            nc.tensor.matmul(
                                ps[:],
                                bqk_sb[0:1, j * P:(j + 1) * P],
                                ones_sb[0:1, :],
                                start=False, stop=True)
                            nc.vector.tensor_copy(qk_sb[:, j, tsl], ps[:])

                        # v[t, j] with per-head stride 65 in vaug
                        for tt in range(4):
                            tglob = tci * 4 + tt
                            ps = ps1v.tile([P, 512], F32, tag="psv")
                            for k in range(KT):
                                nc.tensor.matmul(
                                    ps[:],
                                    xt[:, k, tt * P:(tt + 1) * P],
                                    wv_sb[:, k, :],
                                    start=(k == 0), stop=False)
                            nc.tensor.matmul(
                                ps[:],
                                ones_sb[0:1, 0:P],
                                bv_sb[0:1, :],
                                start=False, stop=True)
                            nc.vector.tensor_copy(
                                vaug[:, tglob, :, 0:HD],
                                ps[:].rearrange("p (h d) -> p h d", h=HPC))

            # ot lives in phases 2+3 only (SBUF too small to coexist with
            # the phase-1 weight/x pools)
            otpool = ctx.enter_context(tc.tile_pool(name="otpool", bufs=1))
            ot_sb = otpool.tile([P, JC // P, T], F32R, tag="ot")          # [128, 4, 2048]

            # ---------------- Phase 2: attention per head ----------------
            with ExitStack() as c2:
                with nc.named_scope("attn"):
                    wtpool = c2.enter_context(tc.tile_pool(name="wtpool", bufs=3))
                    nrmpool = c2.enter_context(tc.tile_pool(name="nrmpool", bufs=2))
                    rbpool = c2.enter_context(tc.tile_pool(name="rbpool", bufs=1))
                    rdpool = c2.enter_context(
                        tc.tile_pool(name="rdpool", bufs=2, space="DRAM"))
                    pss = c2.enter_context(
                        tc.tile_pool(name="pss", bufs=1, space="PSUM"))
                    psav = c2.enter_context(
                        tc.tile_pool(name="psav", bufs=4, space="PSUM"))

                    EXP = mybir.ActivationFunctionType.Exp
                    qpool = c2.enter_context(tc.tile_pool(name="qpool", bufs=2))
                    # two rotating zero-padded qT buffers; heads alternate
                    # parity so each buffer's zero half stays zero
                    qpads = [qpool.tile([P, T], F32R, tag="qpad",
                                        name=f"qpad_{i}") for i in range(2)]
                    for i in range(2):
                        nc.sync.dma_start(qpads[i][:], zeros_d[:])

                    def do_scores(h, k, qpad):
                        jt = h // 2
                        # full 128-row stationary operand (both heads' kT);
                        # the other head's rows hit the zero half of qpad, so
                        # the K=128 contraction equals the K=64 one but keeps
                        # the PE array fully active (HAM stays at 8/8)
                        kT2 = qk_sb[:, jt + 4, :]
                        wt = wtpool.tile([P, T], F32R, tag="wt",
                                         name=f"wt_{h}_{k}")
                        ps = pss.tile([P, 4, 512], F32, tag="ps_s",
                                      name=f"ps_s_{h}_{k}")
                        for c4 in range(4):
                            nc.tensor.matmul(
                                ps[:, c4, :],
                                kT2[:, k * P:(k + 1) * P],
                                qpad[:, c4 * 512:(c4 + 1) * 512],
                                start=True, stop=True)
                        # one maximally wide exp straight from psum
                        nc.scalar.activation(
                            wt[:], ps[:].rearrange("p a b -> p (a b)"),
                            EXP, bias=0.0, scale=0.125)
                        return wt

                    def do_av(h, k, wt, av_tiles):
                        for c4 in range(4):
                            nc.tensor.matmul(
                                av_tiles[c4][:],
                                vaug_f[:, k, h * (HD + 1):h * (HD + 1) + P],
                                wt[:, c4 * 512:(c4 + 1) * 512],
                                start=(k == 0), stop=(k == TT - 1))

                    def finish_head(h, av_tiles):
                        pb = (h % 2) * 64
                        jt = h // 2
                        # free psum fast: copy o rows (unnormalized) + sums row
                        # (sums collect at partition 64, aligned with psum)
                        sums = nrmpool.tile([P, T], F32, tag="sums",
                                            name=f"sums_{h}")
                        for c4 in range(4):
                            csl = slice(c4 * 512, (c4 + 1) * 512)
                            nc.vector.tensor_copy(
                                ot_sb[pb:pb + 64, jt, csl],
                                av_tiles[c4][0:HD, :])
                            nc.vector.tensor_copy(
                                sums[64:65, csl],
                                av_tiles[c4][HD:HD + 1, :])
                        nc.vector.reciprocal(sums[64:65, :], sums[64:65, :])
                        rd = rdpool.tile([1, T], F32, tag="rd",
                                         name=f"rd_{h}")
                        nc.sync.dma_start(rd[:], sums[64:65, :])
                        rb = rbpool.tile([P, T], F32, tag="rb",
                                         name=f"rb_{h}")
                        nc.sync.dma_start(rb[pb:pb + 64, :],
                                          rd[:].to_broadcast((64, T)))
                        nc.vector.tensor_mul(
                            ot_sb[pb:pb + 64, jt, :],
                            ot_sb[pb:pb + 64, jt, :],
                            rb[pb:pb + 64, :])

                    # software pipeline: AV(k-1) is emitted after scores(k) so
                    # the PE always has independent matmuls between exp waits
                    prev = None
                    for h in range(HPC):
                        pb = (h % 2) * 64
                        jt = h // 2
                        qpad = qpads[h % 2]
                        nc.vector.tensor_copy(
                            qpad[pb:pb + HD, :], qk_sb[pb:pb + HD, jt, :])
                        av_tiles = [psav.tile([P, 512], F32, tag="av",
                                              name=f"av_{h}_{i}")
                                    for i in range(4)]
                        for k in range(TT):
                            wt = do_scores(h, k, qpad)
                            if prev is not None:
                                ph, pk, pwt, pav = prev
                                do_av(ph, pk, pwt, pav)
                                if pk == TT - 1:
                                    finish_head(ph, pav)
                            prev = (h, k, wt, av_tiles)
                    ph, pk, pwt, pav = prev
                    do_av(ph, pk, pwt, pav)
                    finish_head(ph, pav)

            # ---------------- Phase 3: out projection ----------------
            with ExitStack() as c3:
                with nc.named_scope("outproj"):
                    wopool = c3.enter_context(tc.tile_pool(name="wopool", bufs=1))
                    opool = c3.enter_context(tc.tile_pool(name="opool", bufs=3))
                    ps3 = c3.enter_context(
                        tc.tile_pool(name="ps3", bufs=4, space="PSUM"))

                    wo_sb = wopool.tile([P, JC // P, D], F32R, tag="wo")
                    nc.sync.dma_start(wo_sb[:], wo_d[:])

                    for tt in range(TT):
                        ost = opool.tile([P, D], F32, tag="ost")
                        for cc in range(2):
                            ps = ps3.tile([P, 512], F32, tag="pso")
                            for jt in range(JC // P):
                                nc.tensor.matmul(
                                    ps[:],
                                    ot_sb[:, jt, tt * P:(tt + 1) * P],
                                    wo_sb[:, jt, cc * 512:(cc + 1) * 512],
                                    start=(jt == 0), stop=(jt == JC // P - 1))
                            nc.vector.tensor_copy(
                                ost[:, cc * 512:(cc + 1) * 512], ps[:])
                        nc.sync.dma_start(out_d[tt * P:(tt + 1) * P, :], ost[:])

    nc.compile()
    return nc


def _prep_core_inputs(x, qkv_w, qkv_b, out_w, core):
    b, g = core // 2, core % 2
    jsl = slice(g * JC, (g + 1) * JC)

    xT = np.ascontiguousarray(x[b].T)                       # [1024, 2048]
    xt = np.ascontiguousarray(
        xT.reshape(KT, P, T).transpose(1, 0, 2))            # [128, 8, 2048]

    Wq = qkv_w[0 * D:1 * D][jsl]                            # [512, 1024]
    Wk = qkv_w[1 * D:2 * D][jsl]
    Wv = qkv_w[2 * D:3 * D][jsl]
    WqkT = np.concatenate([Wq, Wk], axis=0).T               # [1024, 1024]
    wqk = np.ascontiguousarray(
        WqkT.reshape(KT, P, 2 * JC).transpose(1, 0, 2))     # [128, 8, 1024]
    WvT = Wv.T                                              # [1024, 512]
    wv = np.ascontiguousarray(
        WvT.reshape(KT, P, JC).transpose(1, 0, 2))          # [128, 8, 512]

    bqk = np.concatenate(
        [qkv_b[0 * D:1 * D][jsl], qkv_b[1 * D:2 * D][jsl]])[None, :]
    bv = qkv_b[2 * D:3 * D][jsl][None, :]

    WoT = np.ascontiguousarray(out_w[:, jsl].T)             # [512, 1024]
    wo = np.ascontiguousarray(
        WoT.reshape(JC // P, P, D).transpose(1, 0, 2))      # [128, 4, 1024]

    return {
        "xt": xt.astype(np.float32),
        "wqk": wqk.astype(np.float32),
        "wv": wv.astype(np.float32),
        "bqk": bqk.astype(np.float32),
        "bv": bv.astype(np.float32),
        "wo": wo.astype(np.float32),
        "ones": np.ones((P, 512), dtype=np.float32),
        "zeros": np.zeros((P, T), dtype=np.float32),
    }


def run(x, qkv_w, qkv_b, out_w, out_b, trace=False, tmpdir=None):
    if "nc" not in _cached:
        _cached["nc"] = build_program()
    nc = _cached["nc"]
    in_maps = [_prep_core_inputs(x, qkv_w, qkv_b, out_w, c) for c in range(NC)]
    res = run_bass_kernel_spmd(nc, in_maps, core_ids=list(range(NC)),
                               trace=trace, tmpdir=tmpdir)
    parts = np.stack([res.results[c]["out"] for c in range(NC)])  # [8, T, D]
    out = parts.reshape(B, 2, T, D).sum(axis=1) + out_b[None, None, :]
    return out.astype(np.float32), res


def kernel(x, qkv_w, qkv_b, out_w, out_b):
    x = np.asarray(x, dtype=np.float32)
    qkv_w = np.asarray(qkv_w, dtype=np.float32)
    qkv_b = np.asarray(qkv_b, dtype=np.float32)
    out_w = np.asarray(out_w, dtype=np.float32)
    out_b = np.asarray(out_b, dtype=np.float32)
    out, _ = run(x, qkv_w, qkv_b, out_w, out_b, trace=False)
    return out
